# revision 1
# baseline (speedup 1.0000x reference)
"""Trainium2 Bass kernel for nn_AttentionMixer (two-stage grouped attention mixer).

Strategy (per core, data-parallel over batch B=16 -> 2 batches/core):
  - activations kept feature-major ("X^T": [feature, token]) for projections;
    token-major <-> feature-major conversions via DMA xbar transposes (bf16).
  - Q^T, K^T projections feature-major (weights stationary); V projection
    token-major (activation tiles stationary) with an interleaved ones column
    (V_aug) so the AV matmul also produces the softmax denominator.
  - scores computed TRANSPOSED (lhsT=K^T, rhs=Q^T -> [s, l]) so that
    P^T = exp(scale*scores^T) is directly the AV stationary operand and the
    softmax normalizer lands on the PSUM partition axis. No max-subtraction
    (|scaled scores| < ~1.4, validated against the reference).
  - heads are grouped by PE row-group per PSUM bank: base-partition-0 and
    base-partition-64 matmuls never target the same PSUM bank (mixing them
    crashes the exec unit, NRT status 101).
  - all V biases folded host-side into downstream biases (softmax rows sum
    to 1); inter-stage token regroup handled by permuted matmul access
    patterns on the stage-2 projection reads.
Everything bf16 on the PE with fp32 PSUM accumulation.
"""

import numpy as np
import ml_dtypes

import concourse.bass as bass
import concourse.mybir as mybir
import concourse.tile as tile
from concourse import bacc

BF16 = mybir.dt.bfloat16
F32 = mybir.dt.float32
AF = mybir.ActivationFunctionType

D = 512          # d_model
H = 8            # heads
E = 64           # head dim
L = 128          # tokens per attention sequence
NSEQ = 16        # sequences per stage per batch element
NT = 2048        # tokens per batch element
NKT = 4          # 512 // 128 contraction tiles
NB = 2           # batch elements per core
N_CORES = 8
SCALE = 0.125    # 1/sqrt(E)

W_NAMES = ["wq1", "wk1", "wv1", "wq2", "wk2", "wv2", "wo2"]


def _build_kernel(repeat=1, tr_xt=None, tr_b1=None, tr_b2=None):
    import os
    tr_xt = tr_xt or os.environ.get("TR_XT", "pe")
    tr_b1 = tr_b1 or os.environ.get("TR_B1", "pe")
    tr_b2 = tr_b2 or os.environ.get("TR_B2", "dma")
    ps_mm = int(os.environ.get("PS_MM", "4"))
    ps_sc0 = int(os.environ.get("PS_SC0", "1"))
    ps_sc1 = int(os.environ.get("PS_SC1", "1"))
    ps_av = int(os.environ.get("PS_AV", "2"))
    nc = bacc.Bacc("TRN2", target_bir_lowering=False, debug=False)

    x_d = nc.dram_tensor("x", [NB * NT, D], F32, kind="ExternalInput")
    w_d = {n: nc.dram_tensor(n, [D, D], BF16, kind="ExternalInput") for n in W_NAMES}
    qb1_d = nc.dram_tensor("qb1", [128, NKT], F32, kind="ExternalInput")
    kb1_d = nc.dram_tensor("kb1", [128, NKT], F32, kind="ExternalInput")
    qb2_d = nc.dram_tensor("qb2", [128, NKT], F32, kind="ExternalInput")
    kb2_d = nc.dram_tensor("kb2", [128, NKT], F32, kind="ExternalInput")
    v2bc_d = nc.dram_tensor("v2bc", [128, D], F32, kind="ExternalInput")
    o2bc_d = nc.dram_tensor("o2bc", [128, D], F32, kind="ExternalInput")
    out_d = nc.dram_tensor("out", [NB * NT, D], F32, kind="ExternalOutput")

    with tile.TileContext(nc) as tc:
        with (
            tc.tile_pool(name="const", bufs=1) as const_pool,
            tc.tile_pool(name="big", bufs=1) as big,
            tc.tile_pool(name="work", bufs=3) as work,
            tc.tile_pool(name="psum", bufs=2, space="PSUM") as psum,
        ):
            # ---- constants ----
            wsb = {}
            for n in W_NAMES:
                wsb[n] = const_pool.tile([128, NKT * D], BF16, name=f"sb_{n}", tag=f"sb_{n}")
            def load_w(n):
                for ki in range(NKT):
                    eng = nc.sync if ki % 2 == 0 else nc.scalar
                    eng.dma_start(
                        out=wsb[n][:, ki * D:(ki + 1) * D],
                        in_=w_d[n][ki * 128:(ki + 1) * 128, :],
                    )
            load_w("wq1")
            load_w("wk1")
            load_w("wv1")
            biases = {}
            for n, dten in (("qb1", qb1_d), ("kb1", kb1_d), ("qb2", qb2_d), ("kb2", kb2_d)):
                biases[n] = const_pool.tile([128, NKT], F32, name=f"sb_{n}", tag=f"sb_{n}")
                nc.sync.dma_start(out=biases[n][:], in_=dten[:])
            v2bc = const_pool.tile([128, D], F32, name="sb_v2bc", tag="sb_v2bc")
            nc.sync.dma_start(out=v2bc[:], in_=v2bc_d[:])
            o2bc = const_pool.tile([128, D], F32, name="sb_o2bc", tag="sb_o2bc")
            nc.sync.dma_start(out=o2bc[:], in_=o2bc_d[:])
            from concourse.masks import make_identity
            ident32 = const_pool.tile([128, 128], F32, name="ident32", tag="ident32")
            make_identity(nc, ident32)
            ident16 = const_pool.tile([128, 128], BF16, name="ident16", tag="ident16")
            make_identity(nc, ident16)

            def pthcol(h):
                # column block of head h inside the [128,1024] scores/pt tile;
                # row-group-0 heads (even) in bank 0, row-group-64 heads (odd)
                # in bank 1
                return (h % 2) * 512 + (h // 2) * 128

            def projections(rhs_src, lhsT_src, wq, wk, wv, qb, kb, vbias_bc,
                            qt, kt, vaug, pfx):
                """Generator: yields after each Q+K unit / V unit (8 per chunk).
                rhs_src(ki, tc)->[128,512] AP; lhsT_src(ki, tt)->[128,128] AP."""
                vview = vaug.rearrange("p (n h e) -> p n h e", n=NSEQ, h=H)
                nc.vector.memset(vview[:, :, :, E], 1.0)
                for tcn in range(NKT):  # 512-token chunks
                    tsl = slice(tcn * 512, (tcn + 1) * 512)
                    for o in range(NKT):
                        psq = psum.tile([128, 512], F32, name=f"{pfx}q_{tcn}_{o}", tag="mm", bufs=ps_mm)
                        for ki in range(NKT):
                            nc.tensor.matmul(
                                psq[:],
                                lhsT=wq[:, ki * D + o * 128: ki * D + (o + 1) * 128],
                                rhs=rhs_src(ki, tcn),
                                start=(ki == 0), stop=(ki == NKT - 1),
                            )
                        nc.scalar.add(qt[:, o * NT:(o + 1) * NT][:, tsl], psq[:], qb[:, o:o + 1])
                        psk = psum.tile([128, 512], F32, name=f"{pfx}k_{tcn}_{o}", tag="mm", bufs=ps_mm)
                        for ki in range(NKT):
                            nc.tensor.matmul(
                                psk[:],
                                lhsT=wk[:, ki * D + o * 128: ki * D + (o + 1) * 128],
                                rhs=rhs_src(ki, tcn),
                                start=(ki == 0), stop=(ki == NKT - 1),
                            )
                        nc.vector.tensor_scalar_add(
                            kt[:, o * NT:(o + 1) * NT][:, tsl], psk[:], kb[:, o:o + 1])
                        yield
                    for t4 in range(4):  # token-major V per 128-token tile
                        tt = tcn * 4 + t4
                        psv = psum.tile([128, 512], F32, name=f"{pfx}v_{tt}", tag="mm", bufs=ps_mm)
                        for ki in range(NKT):
                            nc.tensor.matmul(
                                psv[:],
                                lhsT=lhsT_src(ki, tt),
                                rhs=wv[:, ki * D:(ki + 1) * D],
                                start=(ki == 0), stop=(ki == NKT - 1),
                            )
                        dst = vview[:, tt, :, 0:E]  # [128, 8, 64] strided
                        psv_r = psv.rearrange("p (h e) -> p h e", h=H)
                        if vbias_bc is None:
                            nc.vector.tensor_copy(dst, psv_r)
                        else:
                            nc.vector.tensor_add(
                                dst, psv_r,
                                vbias_bc.rearrange("p (h e) -> p h e", h=H),
                            )
                        yield

            def attention(qt, kt, vaug, boundary, pfx):
                """One attention stage; boundary(s, htok) emits the
                token->feature-major transpose DMAs for sequence s."""
                for s in range(NSEQ):
                    pssc = [
                        psum.tile([128, 512], F32, name=f"{pfx}sc_{s}_0", tag="sc0", bufs=ps_sc0),
                        psum.tile([128, 512], F32, name=f"{pfx}sc_{s}_1", tag="sc1", bufs=ps_sc1),
                    ]
                    pt = work.tile([128, 1024], BF16, name=f"{pfx}pt_{s}", tag="pt", bufs=int(__import__("os").environ.get("PTB","4")))
                    psav = []
                    for bank in range(2):
                        p = psum.tile([128, 4, E + 1], F32, name=f"{pfx}av_{s}_{bank}", tag="av", bufs=ps_av)
                        psav.append(p)

                    def sc_mm(h):
                        po = 64 * (h % 2)
                        fcol = (h // 2) * NT + s * 128
                        c0 = (h // 2) * 128
                        nc.tensor.matmul(
                            pssc[h % 2][:, c0:c0 + 128],
                            lhsT=kt[po:po + 64, fcol:fcol + 128],
                            rhs=qt[po:po + 64, fcol:fcol + 128],
                            start=True, stop=True,
                        )

                    def av_mm(h):
                        bank, col = h % 2, h // 2
                        c0 = pthcol(h)
                        nc.tensor.matmul(
                            psav[bank][:, col, :],
                            lhsT=pt[:, c0:c0 + 128],
                            rhs=vaug[:, s * (H * (E + 1)) + h * (E + 1):
                                     s * (H * (E + 1)) + (h + 1) * (E + 1)],
                            start=True, stop=True,
                        )

                    # per-bank pipelining: scores(b0), exp(b0), scores(b1)
                    # overlap exp(b0); AV(b0) overlaps exp(b1)
                    for h in (0, 2, 4, 6):
                        sc_mm(h)
                    nc.scalar.activation(pt[:, 0:512], pssc[0][:], AF.Exp, scale=SCALE)
                    for h in (1, 3, 5, 7):
                        sc_mm(h)
                    for h in (0, 2, 4, 6):
                        av_mm(h)
                    nc.scalar.activation(pt[:, 512:1024], pssc[1][:], AF.Exp, scale=SCALE)
                    for h in (1, 3, 5, 7):
                        av_mm(h)
                    htok = work.tile([128, D], BF16, name=f"{pfx}ht_{s}", tag="htok", bufs=int(__import__("os").environ.get("HTB","6")))
                    hv = htok.rearrange("p (c2 b e) -> p c2 b e", b=2, e=E)  # [128,4,2,64]
                    for bank in range(2):
                        rr = work.tile([128, 4], F32, name=f"{pfx}r_{s}_{bank}", tag="rr", bufs=int(__import__("os").environ.get("RRB","2")))
                        nc.vector.reciprocal(rr[:], psav[bank][:, :, E])
                        # head h = 2*col + bank lives at htok cols col*128+bank*64
                        nc.vector.tensor_tensor(
                            hv[:, :, bank, :],
                            psav[bank][:, :, 0:E],
                            rr[:, :, None].broadcast_to((128, 4, E)),
                            mybir.AluOpType.mult,
                        )
                    boundary(s, htok)
                    yield

            def run_iteration_phases(b, it):
                """Build the five phase generators for batch element b."""
                # ---------- load x, cast to bf16, transpose to XT ----------
                xt = big.tile([128, NKT * NT], BF16, name=f"xt_{b}_{it}", tag="xt")

                def gen_xt():
                    for tt in range(NSEQ):
                        xin = work.tile([128, D], F32, name=f"xin_{it}_{tt}", tag="xin", bufs=int(__import__("os").environ.get("XIB","4")))
                        nc.sync.dma_start(
                            out=xin[:], in_=x_d[b * NT + tt * 128: b * NT + (tt + 1) * 128, :])
                        if tr_xt == "dma":
                            xb = work.tile([128, D], BF16, name=f"xb_{it}_{tt}", tag="xb")
                            nc.vector.tensor_copy(xb[:], xin[:])
                            xview = xt.rearrange("p (k t) -> p k t", k=NKT)
                            nc.sync.dma_start(
                                out=xview[:, :, tt * 128:(tt + 1) * 128],
                                in_=xb[:],
                                transpose=True,
                            )
                        else:
                            for ki in range(NKT):
                                pst = psum.tile([128, 128], F32, name=f"pst_{it}_{tt}_{ki}",
                                                tag="mm", bufs=ps_mm)
                                nc.tensor.transpose(pst[:], xin[:, ki * 128:(ki + 1) * 128],
                                                    ident32[:])
                                dst = xt[:, ki * NT + tt * 128: ki * NT + (tt + 1) * 128]
                                if ki % 2 == 0:
                                    nc.vector.tensor_copy(dst, pst[:])
                                else:
                                    nc.scalar.copy(dst, pst[:])
                        yield

                def plain_rhs(src):
                    return lambda ki, tc: src[:, ki * NT + tc * 512: ki * NT + (tc + 1) * 512]

                def plain_lhsT(src):
                    return lambda ki, tt: src[:, ki * NT + tt * 128: ki * NT + (tt + 1) * 128]

                # ---------- stage 1 ----------
                qt1 = big.tile([128, NKT * NT], BF16, name=f"qt1_{it}", tag="qt", bufs=2)
                kt1 = big.tile([128, NKT * NT], BF16, name=f"kt1_{it}", tag="kt", bufs=2)
                va1 = big.tile([128, NSEQ * H * (E + 1)], BF16, name=f"va1_{it}", tag="vaug", bufs=int(__import__("os").environ.get("VAB","2")))
                gen_p1 = projections(plain_rhs(xt), plain_lhsT(xt),
                                     wsb["wq1"], wsb["wk1"], wsb["wv1"],
                                     biases["qb1"], biases["kb1"], None,
                                     qt1, kt1, va1, f"s1p{it}_")

                # h1t is written in STAGE-2 token order (t2 = c*128 + n*8 + p):
                # per stage-1 seq n, a contiguous DMA xbar transpose into a
                # staging tile, then one strided DMA copy scattering columns
                # (c*8+p -> c*128 + n*8 + p). (The xbar transpose itself only
                # supports contiguous outputs on HW.)
                h1t = big.tile([128, NKT * NT], BF16, name=f"h1t_{it}", tag="ht", bufs=2)
                h1v = h1t.rearrange("q (k c nw) -> q k c nw", k=NKT, c=NSEQ)

                def boundary1(s, htok):
                    if tr_b1 == "dma":
                        h1s = work.tile([128, D], BF16, name=f"h1s{it}_{s}", tag="h1s", bufs=3)
                        nc.sync.dma_start(
                            out=h1s.rearrange("q (k t) -> q k t", k=NKT),
                            in_=htok[:],
                            transpose=True,
                        )
                        nc.vector.tensor_copy(
                            h1v[:, :, :, s * 8:(s + 1) * 8],  # [128,4,16,8]
                            h1s.rearrange("q (k c w) -> q k c w", k=NKT, c=NSEQ),
                        )
                    else:
                        for ki in range(NKT):
                            pst = psum.tile([128, 128], BF16, name=f"ptb1{it}_{s}_{ki}",
                                            tag="mm", bufs=ps_mm)
                            nc.tensor.transpose(pst[:], htok[:, ki * 128:(ki + 1) * 128],
                                                ident16[:])
                            dst = h1v[:, ki, :, s * 8:(s + 1) * 8]  # [128,16,8]
                            srcv = pst.rearrange("q (c w) -> q c w", c=NSEQ)
                            if ki % 2 == 0:
                                nc.vector.tensor_copy(dst, srcv)
                            else:
                                nc.scalar.copy(dst, srcv)

                gen_a1 = attention(qt1, kt1, va1, boundary1, f"s1a{it}_")

                # ---------- stage 2 (plain contiguous reads of h1t) ----------
                qt2 = big.tile([128, NKT * NT], BF16, name=f"qt2_{it}", tag="qt", bufs=2)
                kt2 = big.tile([128, NKT * NT], BF16, name=f"kt2_{it}", tag="kt", bufs=2)
                va2 = big.tile([128, NSEQ * H * (E + 1)], BF16, name=f"va2_{it}", tag="vaug", bufs=int(__import__("os").environ.get("VAB","2")))
                gen_p2 = projections(plain_rhs(h1t), plain_lhsT(h1t),
                                     wsb["wq2"], wsb["wk2"], wsb["wv2"],
                                     biases["qb2"], biases["kb2"], v2bc,
                                     qt2, kt2, va2, f"s2p{it}_")

                h2t = big.tile([128, NKT * NT], BF16, name=f"h2t_{it}", tag="ht", bufs=2)
                h2view = h2t.rearrange("p (k t) -> p k t", k=NKT)

                def boundary2(s, htok):
                    if tr_b2 == "dma":
                        nc.sync.dma_start(
                            out=h2view[:, :, s * 128:(s + 1) * 128],
                            in_=htok[:],
                            transpose=True,
                        )
                    else:
                        for ki in range(NKT):
                            pst = psum.tile([128, 128], BF16, name=f"ptb2{it}_{s}_{ki}",
                                            tag="mm", bufs=ps_mm)
                            nc.tensor.transpose(pst[:], htok[:, ki * 128:(ki + 1) * 128],
                                                ident16[:])
                            dst = h2t[:, ki * NT + s * 128: ki * NT + (s + 1) * 128]
                            if ki % 2 == 0:
                                nc.vector.tensor_copy(dst, pst[:])
                            else:
                                nc.scalar.copy(dst, pst[:])

                gen_a2 = attention(qt2, kt2, va2, boundary2, f"s2a{it}_")

                out_v = out_d.rearrange("(bb n c p) d -> bb c n p d", bb=NB, n=NSEQ, c=NSEQ)

                def gen_out2():
                    for tt in range(NSEQ):  # stage-2 seq index c
                        pso = psum.tile([128, 512], F32, name=f"o2_{it}_{tt}", tag="mm", bufs=ps_mm)
                        for ki in range(NKT):
                            nc.tensor.matmul(
                                pso[:],
                                lhsT=h2t[:, ki * NT + tt * 128: ki * NT + (tt + 1) * 128],
                                rhs=wsb["wo2"][:, ki * D:(ki + 1) * D],
                                start=(ki == 0), stop=(ki == NKT - 1),
                            )
                        osb = work.tile([128, D], F32, name=f"osb_{it}_{tt}", tag="osb", bufs=int(__import__("os").environ.get("OSB","4")))
                        nc.vector.tensor_add(osb[:], pso[:], o2bc[:])
                        nc.sync.dma_start(out=out_v[b, tt], in_=osb[:])
                        yield

                def chain2(g1, g2):
                    yield from g1
                    yield from g2

                def gen_xp1():
                    gx = gen_xt()
                    for _ in range(int(os.environ.get("XLEAD", "4"))):
                        next(gx, None)
                    gx_alive = p_alive = True
                    while gx_alive or p_alive:
                        if p_alive:
                            p_alive = next(gen_p1, SENT) is not SENT
                        if p_alive:
                            p_alive = next(gen_p1, SENT) is not SENT
                        if gx_alive:
                            gx_alive = next(gx, SENT) is not SENT
                        yield

                return {
                    "xp1": gen_xp1(),
                    "a1": gen_a1,
                    "p2": gen_p2,
                    "a2": gen_a2,
                    "o2": gen_out2(),
                }

            def drain(g):
                for _ in g:
                    pass

            def chain_g(g1, g2):
                yield from g1
                yield from g2

            def take_g(g, n):
                for _ in range(n):
                    if next(g, SENT) is SENT:
                        return
                    yield

            ILR = int(os.environ.get("ILR", "2"))

            def interleave(ga, gp):
                """Alternate attention/projection units, ILR proj per attn."""
                a_alive = p_alive = True
                while a_alive or p_alive:
                    for _ in range(ILR):
                        if p_alive:
                            p_alive = next(gp, SENT) is not SENT
                    if a_alive:
                        a_alive = next(ga, SENT) is not SENT

            SENT = object()
            list_len = {}

            # staggered software pipeline over the NB*repeat iterations:
            # attention phases (ACT/DVE-heavy, PE-light) interleave with the
            # next independent projection phases (PE-dense).
            iters = [bb % NB for bb in range(NB * repeat)]
            ph = [run_iteration_phases(b_, i_) for i_, b_ in enumerate(iters)]
            for pair_start in range(0, len(ph), 2):
                P0 = ph[pair_start]
                P1 = ph[pair_start + 1]
                drain(P0["xp1"])
                if pair_start == 0:
                    for n_ in ("wq2", "wk2", "wv2", "wo2"):
                        load_w(n_)
                interleave(P0["a1"], P1["xp1"])
                import os as _os
                _t1 = int(_os.environ.get("TK1", "32"))
                _t2 = int(_os.environ.get("TK2", "4"))
                interleave(P1["a1"], take_g(P0["p2"], _t1))
                interleave(P0["a2"], chain_g(P0["p2"], take_g(P1["p2"], _t2)))
                interleave(P1["a2"], chain_g(P1["p2"], P0["o2"]))
                drain(P1["o2"])

    nc.compile()
    return nc


_NC_CACHE = {}


def _get_nc(repeat=1):
    import os
    key = ("nc", repeat, os.environ.get("ILR", "2"), os.environ.get("XLEAD", "4"),
           os.environ.get("TK1", "32"), os.environ.get("TK2", "4"),
           os.environ.get("PTB", "4"), os.environ.get("HTB", "6"),
           os.environ.get("RRB", "2"), os.environ.get("XIB", "4"),
           os.environ.get("OSB", "4"), os.environ.get("VAB", "2"),
           os.environ.get("TR_XT", "pe"),
           os.environ.get("TR_B1", "pe"), os.environ.get("TR_B2", "dma"),
           os.environ.get("PS_MM", "4"), os.environ.get("PS_SC0", "1"),
           os.environ.get("PS_SC1", "1"), os.environ.get("PS_AV", "2"))
    if key not in _NC_CACHE:
        _NC_CACHE[key] = _build_kernel(repeat)
    return _NC_CACHE[key]


def _prep_inputs(inputs):
    """Host-side data prep: shard x over cores, transpose+cast weights,
    fold V biases into downstream biases."""
    bf = ml_dtypes.bfloat16
    f32 = np.float32
    x = np.ascontiguousarray(np.asarray(inputs["x"], dtype=f32))  # [16,256,8,512]
    B = x.shape[0]
    xs = x.reshape(B, 256 * 8, D)

    g = {k: np.asarray(v, dtype=f32) for k, v in inputs.items() if k != "x"}
    q2_eb = g["q2_w"] @ g["v1_b"] + g["q2_b"]
    k2_eb = g["k2_w"] @ g["v1_b"] + g["k2_b"]
    v2_eb = g["v2_w"] @ g["v1_b"]
    o2_eb = g["out2_w"] @ g["v2_b"] + g["out2_b"]

    def wt(w):
        return np.ascontiguousarray(w.astype(bf).T)

    def btile(v):
        return np.ascontiguousarray(v.reshape(NKT, 128).T.astype(f32))

    common = {
        "wq1": wt(g["q1_w"]), "wk1": wt(g["k1_w"]), "wv1": wt(g["v1_w"]),
        "wq2": wt(g["q2_w"]), "wk2": wt(g["k2_w"]), "wv2": wt(g["v2_w"]),
        "wo2": wt(g["out2_w"]),
        "qb1": btile(g["q1_b"]), "kb1": btile(g["k1_b"]),
        "qb2": btile(q2_eb), "kb2": btile(k2_eb),
        "v2bc": np.ascontiguousarray(np.tile(v2_eb.astype(f32), (128, 1))),
        "o2bc": np.ascontiguousarray(np.tile(o2_eb.astype(f32), (128, 1))),
    }
    in_maps = []
    for c in range(N_CORES):
        m = dict(common)
        m["x"] = np.ascontiguousarray(
            xs[c * NB:(c + 1) * NB].reshape(NB * NT, D))
        in_maps.append(m)
    return in_maps


def _get_executor(repeat=1):
    """Build (once) a jitted shard_map executor over the 8 cores.

    Returns run(in_maps) -> list of per-core out arrays. Mirrors
    bass2jax.run_bass_via_pjrt but caches the jitted callable so repeat
    invocations don't retrace/recompile."""
    key = ("exec", repeat)
    if key in _NC_CACHE:
        return _NC_CACHE[key]

    import jax
    import concourse.mybir as mb
    from jax.sharding import Mesh, PartitionSpec
    from jax.experimental.shard_map import shard_map
    from concourse.bass2jax import (
        _bass_exec_p, install_neuronx_cc_hook, partition_id_tensor,
    )

    nc = _get_nc(repeat)
    install_neuronx_cc_hook()

    partition_name = nc.partition_id_tensor.name if nc.partition_id_tensor else None
    in_names = []
    out_names = []
    out_avals = []
    for alloc in nc.m.functions[0].allocations:
        if not isinstance(alloc, mb.MemoryLocationSet):
            continue
        name = alloc.memorylocations[0].name
        if alloc.kind == "ExternalInput":
            if name != partition_name:
                in_names.append(name)
        elif alloc.kind == "ExternalOutput":
            shape = tuple(alloc.tensor_shape)
            dtype = mb.dt.np(alloc.dtype)
            out_names.append(name)
            out_avals.append(jax.core.ShapedArray(shape, dtype))
    n_params = len(in_names)
    all_names = in_names + out_names
    if partition_name is not None:
        all_names = all_names + [partition_name]

    def _body(*args):
        operands = list(args)
        if partition_name is not None:
            operands.append(partition_id_tensor())
        outs = _bass_exec_p.bind(
            *operands,
            out_avals=tuple(out_avals),
            in_names=tuple(all_names),
            out_names=tuple(out_names),
            lowering_input_output_aliases=(),
            sim_require_finite=True,
            sim_require_nnan=True,
            nc=nc,
        )
        return tuple(outs)

    devices = jax.devices()[:N_CORES]
    mesh = Mesh(np.asarray(devices), ("core",))
    n_outs = len(out_names)
    sharded = jax.jit(
        shard_map(
            _body, mesh=mesh,
            in_specs=(PartitionSpec("core"),) * (n_params + n_outs),
            out_specs=(PartitionSpec("core"),) * n_outs,
            check_rep=False,
        ),
        keep_unused=True,
    )
    zero_outs = [np.zeros((N_CORES * a.shape[0], *a.shape[1:]), a.dtype)
                 for a in out_avals]

    def run(in_maps):
        concat_in = [
            np.concatenate([np.asarray(in_maps[c][nm]) for c in range(N_CORES)], axis=0)
            for nm in in_names
        ]
        out_arrs = sharded(*concat_in, *zero_outs)
        out = np.asarray(out_arrs[0])
        return [out.reshape(N_CORES, *out_avals[0].shape)[c] for c in range(N_CORES)]

    _NC_CACHE[key] = (run, sharded, in_names, zero_outs)
    return _NC_CACHE[key]


def run_kernel_results(inputs, trace=False):
    run = _get_executor()[0]
    in_maps = _prep_inputs(inputs)
    outs = run(in_maps)
    full = np.concatenate(
        [r.reshape(NB, 256, 8, D) for r in outs], axis=0).astype(np.float32)
    return full, None


def kernel(**inputs):
    full, _ = run_kernel_results(inputs)
    return full

